# revision 11
# baseline (speedup 1.0000x reference)
"""Block-causal attention (B=2, S=2048, D=1024, H=16, HD=64, BLOCK=16) on 8 TRN2 cores.

Sharding: core c -> batch c//4, head-group c%4 (4 heads). Each core computes the
full attention for its 4 heads plus a partial out-projection y^T (1024, 2048);
the host sums the 4 partials per batch (row-parallel unshard) and transposes.

Device dataflow (per core) is fully "transposed":
  - qkv proj emits q^T/k^T in (head-dim, seq) layout, V in (seq, head-dim).
  - RMS-norm on q^T/k^T: squares on ACT, partition-sum via ones-matmul on PE,
    rsqrt as exp(-0.5*ln(.)) so Ln/Exp/Square share one ACT table set.
  - scores^T = (K^T-tile).T @ Q^T per head; block-causal mask is added inside
    the PE accumulation as a rank-8 (-720 * disallowed) matmul; exp needs no
    row-max because |scores| <= 8 after RMS norm (host passes the bound).
  - softmax denominator comes free: V carries an appended ones column (M=65).
  - attn^T = [V|1].T @ P^T accumulated over k-tiles; normalize by the
    reciprocal of row 64.
"""

import numpy as np
import ml_dtypes

import concourse.bass as bass
import concourse.tile as tile
from concourse import bacc
from concourse import mybir
from concourse.bass_utils import run_bass_kernel_spmd

BF16 = ml_dtypes.bfloat16
F32 = mybir.dt.float32
BF = mybir.dt.bfloat16

B, S, D, H, HD = 2, 2048, 1024, 16, 64
HLOC = 4          # heads per core
NCORES = 8
EPS = 1e-6
SCALE = HD ** -0.5
MASK_C = 720.0    # score offset for masked pairs: exp(scale*(s-720)-B0) == 0.0
NST = 4           # 512-wide seq tiles
NKT = 16          # 128-wide key tiles
NDK = 8           # 128-wide model-dim tiles


def _emit(tc):
    """Emit the per-core program. Pure SPMD: identical on all 8 cores."""
    from contextlib import ExitStack

    nc = tc.nc
    A = mybir.ActivationFunctionType
    OP = mybir.AluOpType

    def din(name, shape, d=BF):
        return nc.dram_tensor(name, shape, d, kind="ExternalInput").ap()

    xt_d = din("xt", [D, S])
    wq_d = din("wq", [D, 256])
    wk_d = din("wk", [D, 256])
    wv_d = din("wv", [D, 256])
    wo_d = din("wo", [256, D])
    cs_d = din("cs", [128, S])
    sn_d = din("sn", [128, S])
    qn_d = din("qn2", [128, 1], F32)
    kn_d = din("kn2", [128, 1], F32)
    mu_d = din("mu", [8, 128])
    mv_d = din("mv", [8, 128])
    ones2_d = din("ones2", [128, 2])
    b0_d = din("b0", [128, 1], F32)
    yt_d = nc.dram_tensor("yt", [D, S], F32, kind="ExternalOutput").ap()

    ctx = ExitStack()
    proj_ctx = ExitStack()
    with ctx:
        consts = ctx.enter_context(tc.tile_pool(name="consts", bufs=1))
        persist = ctx.enter_context(tc.tile_pool(name="persist", bufs=1))
        dscratch = ctx.enter_context(tc.tile_pool(name="dscratch", bufs=1, space="DRAM"))
        xtp = proj_ctx.enter_context(tc.tile_pool(name="xtp", bufs=1))
        work2 = proj_ctx.enter_context(tc.tile_pool(name="work2", bufs=2))
        sqp = proj_ctx.enter_context(tc.tile_pool(name="sqp", bufs=3))
        pp = proj_ctx.enter_context(tc.tile_pool(name="pp", bufs=2, space="PSUM"))
        vp = proj_ctx.enter_context(tc.tile_pool(name="vp", bufs=2, space="PSUM"))
        msp = proj_ctx.enter_context(tc.tile_pool(name="msp", bufs=1, space="PSUM"))

        # ---- constant / weight loads ----
        wq_sb = consts.tile([128, NDK, 256], BF)
        wk_sb = consts.tile([128, NDK, 256], BF)
        wv_sb = consts.tile([128, NDK, 256], BF)
        wo_sb = consts.tile([128, 2, D], BF)
        nc.sync.dma_start(out=wq_sb, in_=wq_d.rearrange("(t p) m -> p t m", p=128))
        nc.sync.dma_start(out=wk_sb, in_=wk_d.rearrange("(t p) m -> p t m", p=128))
        nc.sync.dma_start(out=wv_sb, in_=wv_d.rearrange("(t p) m -> p t m", p=128))
        nc.sync.dma_start(out=wo_sb, in_=wo_d.rearrange("(t p) m -> p t m", p=128))
        cs_sb = consts.tile([128, S], BF)
        sn_sb = consts.tile([128, S], BF)
        nc.sync.dma_start(out=cs_sb, in_=cs_d)
        nc.sync.dma_start(out=sn_sb, in_=sn_d)
        qn_sb = consts.tile([128, 1], F32)
        kn_sb = consts.tile([128, 1], F32)
        nc.sync.dma_start(out=qn_sb, in_=qn_d)
        nc.sync.dma_start(out=kn_sb, in_=kn_d)
        mu_sb = consts.tile([8, 128], BF)
        mv_sb = consts.tile([8, 128], BF)
        nc.sync.dma_start(out=mu_sb, in_=mu_d)
        nc.sync.dma_start(out=mv_sb, in_=mv_d)
        ones2_sb = consts.tile([128, 2], BF)
        nc.sync.dma_start(out=ones2_sb, in_=ones2_d)
        b0_sb = consts.tile([128, 1], F32)
        nc.sync.dma_start(out=b0_sb, in_=b0_d)
        eps_sb = consts.tile([128, 1], F32)
        nc.vector.memset(eps_sb, EPS)

        xt_sb = xtp.tile([128, NDK, S], BF)
        for kt in range(NDK):
            nc.sync.dma_start(
                out=xt_sb[:, kt, :], in_=xt_d[128 * kt : 128 * (kt + 1), :]
            )

        # ---- persistent activations ----
        qT = persist.tile([128, 2, S], BF)      # (2 heads)*64 rows per m-tile
        kT = persist.tile([128, 2, S], BF)
        vv = persist.tile([128, NKT, HLOC, HD + 1], BF)   # [V | ones]
        at = persist.tile([128, 2, S], BF)      # normalized attn^T
        # pair p's two rows live at partition 32*p (engines need 32-aligned
        # start partitions)
        ln8 = persist.tile([98, NST, 512], F32)
        rr8 = persist.tile([98, NST, 512], BF)
        rr_dram = dscratch.tile([8, NST, 512], BF)
        den_dram = dscratch.tile([4, NST, 512], F32)

        nc.vector.memset(vv[:, :, :, HD : HD + 1], 1.0)

        # ---- phase 1: V projection (s, d) ----
        for st in range(NKT):  # 128-row seq tiles
            ps = vp.tile([128, 256], F32, tag="vp")
            for kt in range(NDK):
                nc.tensor.matmul(
                    ps,
                    lhsT=xt_sb[:, kt, 128 * st : 128 * (st + 1)],
                    rhs=wv_sb[:, kt, :],
                    start=(kt == 0),
                    stop=(kt == NDK - 1),
                )
            nc.vector.tensor_copy(
                vv[:, st, :, 0:HD], ps.rearrange("p (h d) -> p h d", h=HLOC)
            )

        # ---- phase 2: Q/K projection + RMS norm + RoPE, (d, s) layout ----
        for qk_i, (wsb, nsb, dest) in enumerate(
            [(wq_sb, qn_sb, qT), (wk_sb, kn_sb, kT)]
        ):
            for mt in range(2):
                pair = qk_i * 2 + mt
                qraw = work2.tile([128, S], BF, tag="qraw")
                ms = msp.tile([2, NST, 512], F32, tag="msp")
                for st in range(NST):
                    ps = pp.tile([128, 512], F32, tag="pp")
                    for kt in range(NDK):
                        nc.tensor.matmul(
                            ps,
                            lhsT=wsb[:, kt, 128 * mt : 128 * (mt + 1)],
                            rhs=xt_sb[:, kt, 512 * st : 512 * (st + 1)],
                            start=(kt == 0),
                            stop=(kt == NDK - 1),
                        )
                    sl = slice(512 * st, 512 * (st + 1))
                    nc.vector.tensor_copy(qraw[:, sl], ps)
                    sq = sqp.tile([128, 512], BF, tag="sq")
                    nc.scalar.activation(sq, qraw[:, sl], A.Square)
                    nc.tensor.matmul(
                        ms[:, st, :], lhsT=ones2_sb, rhs=sq, start=True, stop=True
                    )
                # rrms = exp(-0.5 * ln(ms/HD + eps)) for the 2 heads of this pair
                pb = 32 * pair
                nc.scalar.activation(
                    ln8[pb : pb + 2],
                    ms,
                    A.Ln,
                    bias=eps_sb[0:2],
                    scale=1.0 / HD,
                )
                nc.scalar.activation(
                    rr8[pb : pb + 2],
                    ln8[pb : pb + 2],
                    A.Exp,
                    scale=-0.5,
                )
                # broadcast rrms rows across the 64 partitions of each head
                rrb = work2.tile([128, NST, 512], BF, tag="rrb")
                nc.sync.dma_start(
                    out=rr_dram[2 * pair : 2 * pair + 2],
                    in_=rr8[pb : pb + 2],
                )
                nc.gpsimd.dma_start(
                    out=rrb[0:64],
                    in_=rr_dram[2 * pair : 2 * pair + 1].partition_broadcast(64),
                )
                nc.gpsimd.dma_start(
                    out=rrb[64:128],
                    in_=rr_dram[2 * pair + 1 : 2 * pair + 2].partition_broadcast(64),
                )
                # qhat = (qraw * qn_w) * rrms   (normalize + weight, bf16)
                qhat = work2.tile([128, S], BF, tag="qhat")
                for st in range(NST):
                    sl = slice(512 * st, 512 * (st + 1))
                    nc.vector.scalar_tensor_tensor(
                        qhat[:, sl],
                        qraw[:, sl],
                        nsb,
                        rrb[:, st, :],
                        op0=OP.mult,
                        op1=OP.mult,
                    )
                # RoPE: dest = qhat*cos + rot(qhat)*sn  (sn sign-folded on host)
                rot = work2.tile([128, S], BF, tag="rot")
                for lo, hi in ((0, 32), (32, 64), (64, 96), (96, 128)):
                    src_lo = lo + 32 if (lo // 32) % 2 == 0 else lo - 32
                    nc.sync.dma_start(
                        out=rot[lo:hi], in_=qhat[src_lo : src_lo + 32]
                    )
                t1 = work2.tile([128, S], BF, tag="t1")
                t2 = work2.tile([128, S], BF, tag="t2")
                nc.vector.tensor_mul(t1, qhat, cs_sb)
                nc.vector.tensor_mul(t2, rot, sn_sb)
                nc.vector.tensor_add(dest[:, mt, :], t1, t2)

        # proj scratch (incl. x^T) is dead now; free SBUF/PSUM for attention
        proj_ctx.close()
        attn_ctx = ExitStack()
        attnw = ctx.enter_context(tc.tile_pool(name="attnw", bufs=2))
        ptp = ctx.enter_context(tc.tile_pool(name="ptp", bufs=3))
        ystp = ctx.enter_context(tc.tile_pool(name="ystp", bufs=3))
        spp = attn_ctx.enter_context(tc.tile_pool(name="spp", bufs=2, space="PSUM"))
        avp = attn_ctx.enter_context(tc.tile_pool(name="avp", bufs=1, space="PSUM"))

        # ---- phase 3: attention, one head at a time ----
        for h in range(HLOC):
            mt, half = divmod(h, 2)
            po = 64 * half
            av = avp.tile([65, NST, 512], F32, tag="avp")
            for i in range(NKT):
                q0 = 128 * i  # first unmasked query column for this k-tile
                pt = ptp.tile([128, S], BF, tag="pt")
                for g in range(i // 8, 2):
                    sp = spp.tile([128, 1024], F32, tag="spp")
                    glo = 1024 * g
                    has_diag = glo <= q0 < glo + 1024  # only for g == i//8
                    for jj in range(2):
                        j = 2 * g + jj
                        lo = max(512 * j, q0)
                        hi = 512 * (j + 1)
                        if lo >= hi:
                            continue
                        diag_bank = has_diag and (q0 - glo) // 512 == jj
                        nc.tensor.matmul(
                            sp[:, lo - glo : hi - glo],
                            lhsT=kT[po : po + 64, mt, 128 * i : 128 * (i + 1)],
                            rhs=qT[po : po + 64, mt, lo:hi],
                            start=True,
                            stop=not diag_bank,
                        )
                        if diag_bank:
                            # block-causal mask: scores -= 720*disallowed
                            nc.tensor.matmul(
                                sp[:, q0 - glo : q0 - glo + 128],
                                lhsT=mu_sb,
                                rhs=mv_sb,
                                start=False,
                                stop=True,
                            )
                    lo = max(glo, q0)
                    nc.scalar.activation(
                        pt[:, lo : glo + 1024],
                        sp[:, lo - glo : 1024],
                        A.Exp,
                        bias=b0_sb,
                        scale=SCALE,
                    )
                # attn^T accumulation over k-tiles (+ denominator row 64)
                for j in range(i // 4, NST):
                    lo = max(512 * j, q0)
                    hi = 512 * (j + 1)
                    nc.tensor.matmul(
                        av[:, j, lo - 512 * j : 512],
                        lhsT=vv[:, i, h, :],
                        rhs=pt[:, lo:hi],
                        start=(i == 0),
                        stop=(i == 4 * j + 3),
                    )
            # normalize: at[head rows] = av[0:64] * (1 / av[64])
            rden = attnw.tile([1, NST, 512], F32, tag="rden")
            nc.vector.reciprocal(rden, av[64:65])
            nc.sync.dma_start(out=den_dram[h : h + 1], in_=rden)
            rdb = attnw.tile([64, NST, 512], F32, tag="rdb")
            nc.gpsimd.dma_start(
                out=rdb, in_=den_dram[h : h + 1].partition_broadcast(64)
            )
            for j in range(NST):
                nc.vector.tensor_mul(
                    at[po : po + 64, mt, 512 * j : 512 * (j + 1)],
                    av[0:64, j, :],
                    rdb[:, j, :],
                )

        # ---- phase 4: partial out-projection y^T = wo^T @ at ----
        attn_ctx.close()
        pp = ctx.enter_context(tc.tile_pool(name="pp2", bufs=3, space="PSUM"))
        for m in range(8):
            for j in range(NST):
                ps = pp.tile([128, 512], F32, tag="pp")
                for kt in range(2):
                    nc.tensor.matmul(
                        ps,
                        lhsT=wo_sb[:, kt, 128 * m : 128 * (m + 1)],
                        rhs=at[:, kt, 512 * j : 512 * (j + 1)],
                        start=(kt == 0),
                        stop=(kt == 1),
                    )
                yst = ystp.tile([128, 512], F32, tag="yst")
                if (m + j) % 2 == 0:
                    nc.vector.tensor_copy(yst, ps)
                else:
                    nc.scalar.copy(yst, ps)
                nc.sync.dma_start(
                    out=yt_d[128 * m : 128 * (m + 1), 512 * j : 512 * (j + 1)],
                    in_=yst,
                )


def build_program():
    nc = bacc.Bacc(
        "TRN2",
        target_bir_lowering=False,
        debug=False,
        enable_asserts=False,
        num_devices=NCORES,
    )
    with tile.TileContext(nc) as tc:
        _emit(tc)
    nc.compile()
    return nc


def make_core_inputs(x, qkv_w, out_w, qn_w, kn_w, rope_cos, rope_sin, attention_mask):
    """Host-side shard/layout prep. Returns list of 8 per-core input dicts."""
    x = np.asarray(x, np.float32)
    qkv_w = np.asarray(qkv_w, np.float32)
    out_w = np.asarray(out_w, np.float32)
    qn_w = np.asarray(qn_w, np.float32)
    kn_w = np.asarray(kn_w, np.float32)
    rope_cos = np.asarray(rope_cos, np.float32)
    rope_sin = np.asarray(rope_sin, np.float32)
    am = np.asarray(attention_mask)

    r = qkv_w.reshape(3, H, HD, D)
    csT = rope_cos.T.astype(BF16)                      # (64, S)
    snT = rope_sin.T.astype(np.float32)
    s2 = np.concatenate([-snT[0:32], snT[32:64]], axis=0).astype(BF16)
    cs2 = np.concatenate([csT, csT], axis=0)           # (128, S)
    sn2 = np.concatenate([s2, s2], axis=0)
    qn2 = np.tile(qn_w, 2)[:, None].astype(np.float32)
    kn2 = np.tile(kn_w, 2)[:, None].astype(np.float32)

    # rank-8 factorization of the (128,128) diagonal-block mask
    dis = ~(am[0:128, 0:128].T)                        # dis[k', q'] disallowed
    mu = np.zeros((8, 128), np.float32)
    mv = np.zeros((8, 128), np.float32)
    for t in range(8):
        mu[t] = np.arange(128) // 16 == t
        mv[t] = -MASK_C * dis[16 * t, :]
    ones2 = np.zeros((128, 2), np.float32)
    ones2[0:64, 0] = 1.0
    ones2[64:128, 1] = 1.0
    b0 = float(HD * SCALE * max(1e-30, np.abs(qn_w).max() * np.abs(kn_w).max()))
    b0_t = np.full((128, 1), -b0, np.float32)

    shared = dict(
        cs=cs2.astype(BF16),
        sn=sn2.astype(BF16),
        qn2=qn2,
        kn2=kn2,
        mu=mu.astype(BF16),
        mv=mv.astype(BF16),
        ones2=ones2.astype(BF16),
        b0=b0_t,
    )
    in_maps = []
    for c in range(NCORES):
        b, g = divmod(c, 4)
        hs = slice(HLOC * g, HLOC * (g + 1))
        m = dict(shared)
        m["xt"] = np.ascontiguousarray(x[b].T).astype(BF16)
        m["wq"] = np.ascontiguousarray(
            r[0, hs].transpose(2, 0, 1).reshape(D, 256)
        ).astype(BF16)
        m["wk"] = np.ascontiguousarray(
            r[1, hs].transpose(2, 0, 1).reshape(D, 256)
        ).astype(BF16)
        m["wv"] = np.ascontiguousarray(
            r[2, hs].transpose(2, 0, 1).reshape(D, 256)
        ).astype(BF16)
        m["wo"] = np.ascontiguousarray(
            out_w[:, 256 * g : 256 * (g + 1)].T
        ).astype(BF16)
        in_maps.append(m)
    return in_maps


_PROGRAM = []


def get_program():
    if not _PROGRAM:
        _PROGRAM.append(build_program())
    return _PROGRAM[0]


def unshard(results):
    """results: list of 8 dicts with 'yt' (1024, 2048) fp32 partials."""
    ys = []
    for b in range(B):
        acc = np.zeros((D, S), np.float64)
        for g in range(4):
            acc += np.asarray(results[4 * b + g]["yt"], np.float32)
        ys.append(acc.T.astype(np.float32))
    return np.stack(ys)


def kernel(**inputs):
    in_maps = make_core_inputs(**inputs)
    nc = get_program()
    res = run_bass_kernel_spmd(nc, in_maps, core_ids=list(range(NCORES)))
    return unshard(res.results)


# revision 13
# speedup vs baseline: 23.6841x; 23.6841x over previous
"""Block-causal attention (B=2, S=2048, D=1024, H=16, HD=64, BLOCK=16) on 8 TRN2 cores.

Sharding: core c -> batch c//4, head-group c%4 (4 heads). Each core computes the
full attention for its 4 heads plus a partial out-projection y^T (1024, 2048);
the host sums the 4 partials per batch (row-parallel unshard) and transposes.

Device dataflow (per core) is fully "transposed":
  - qkv proj emits q^T/k^T in (head-dim, seq) layout, V in (seq, head-dim).
  - RMS-norm on q^T/k^T: squares on ACT, partition-sum via ones-matmul on PE,
    rsqrt as exp(-0.5*ln(.)) so Ln/Exp/Square share one ACT table set.
  - scores^T = (K^T-tile).T @ Q^T per head; block-causal mask is added inside
    the PE accumulation as a rank-8 (-720 * disallowed) matmul; exp needs no
    row-max because |scores| <= 8 after RMS norm (host passes the bound).
  - softmax denominator comes free: V carries an appended ones column (M=65).
  - attn^T = [V|1].T @ P^T accumulated over k-tiles; normalize by the
    reciprocal of row 64.
"""

import numpy as np
import ml_dtypes

import concourse.bass as bass
import concourse.tile as tile
from concourse import bacc
from concourse import mybir
from concourse.bass_utils import run_bass_kernel_spmd

BF16 = ml_dtypes.bfloat16
F32 = mybir.dt.float32
BF = mybir.dt.bfloat16

B, S, D, H, HD = 2, 2048, 1024, 16, 64
HLOC = 4          # heads per core
NCORES = 8
EPS = 1e-6
SCALE = HD ** -0.5
MASK_C = 720.0    # score offset for masked pairs: exp(scale*(s-720)-B0) == 0.0
NST = 4           # 512-wide seq tiles
NKT = 16          # 128-wide key tiles
NDK = 8           # 128-wide model-dim tiles


def _emit(tc):
    """Emit the per-core program. Pure SPMD: identical on all 8 cores."""
    from contextlib import ExitStack

    nc = tc.nc
    A = mybir.ActivationFunctionType
    OP = mybir.AluOpType

    def din(name, shape, d=BF):
        return nc.dram_tensor(name, shape, d, kind="ExternalInput").ap()

    xt_d = din("xt", [D, S])
    wq_d = din("wq", [128, NDK * 256])
    wk_d = din("wk", [128, NDK * 256])
    wv_d = din("wv", [128, NDK * 256])
    wo_d = din("wo", [128, 2 * D])
    cs_d = din("cs", [128, S])
    sn_d = din("sn", [128, S])
    qn_d = din("qn2", [128, 1], F32)
    kn_d = din("kn2", [128, 1], F32)
    mu_d = din("mu", [8, 128])
    mv_d = din("mv", [8, 128])
    ones2_d = din("ones2", [128, 2])
    b0_d = din("b0", [128, 1], F32)
    yt_d = nc.dram_tensor("yt", [32, 128, 512], F32, kind="ExternalOutput").ap()

    ctx = ExitStack()
    proj_ctx = ExitStack()
    with ctx:
        consts = ctx.enter_context(tc.tile_pool(name="consts", bufs=1))
        persist = ctx.enter_context(tc.tile_pool(name="persist", bufs=1))
        dscratch = ctx.enter_context(tc.tile_pool(name="dscratch", bufs=1, space="DRAM"))
        xtp = proj_ctx.enter_context(tc.tile_pool(name="xtp", bufs=1))
        work2 = proj_ctx.enter_context(tc.tile_pool(name="work2", bufs=2))
        sqp = proj_ctx.enter_context(tc.tile_pool(name="sqp", bufs=3))
        pp = proj_ctx.enter_context(tc.tile_pool(name="pp", bufs=2, space="PSUM"))
        vp = proj_ctx.enter_context(tc.tile_pool(name="vp", bufs=2, space="PSUM"))
        msp = proj_ctx.enter_context(tc.tile_pool(name="msp", bufs=1, space="PSUM"))

        # ---- constant / weight loads ----
        wq_sb = consts.tile([128, NDK, 256], BF)
        wk_sb = consts.tile([128, NDK, 256], BF)
        wv_sb = consts.tile([128, NDK, 256], BF)
        wo_sb = consts.tile([128, 2, D], BF)
        nc.sync.dma_start(out=wq_sb, in_=wq_d.rearrange("p (t m) -> p t m", t=NDK))
        nc.sync.dma_start(out=wk_sb, in_=wk_d.rearrange("p (t m) -> p t m", t=NDK))
        nc.sync.dma_start(out=wv_sb, in_=wv_d.rearrange("p (t m) -> p t m", t=NDK))
        nc.sync.dma_start(out=wo_sb, in_=wo_d.rearrange("p (t m) -> p t m", t=2))
        cs_sb = consts.tile([128, S], BF)
        sn_sb = consts.tile([128, S], BF)
        nc.sync.dma_start(out=cs_sb, in_=cs_d)
        nc.sync.dma_start(out=sn_sb, in_=sn_d)
        qn_sb = consts.tile([128, 1], F32)
        kn_sb = consts.tile([128, 1], F32)
        nc.sync.dma_start(out=qn_sb, in_=qn_d)
        nc.sync.dma_start(out=kn_sb, in_=kn_d)
        mu_sb = consts.tile([8, 128], BF)
        mv_sb = consts.tile([8, 128], BF)
        nc.sync.dma_start(out=mu_sb, in_=mu_d)
        nc.sync.dma_start(out=mv_sb, in_=mv_d)
        ones2_sb = consts.tile([128, 2], BF)
        nc.sync.dma_start(out=ones2_sb, in_=ones2_d)
        b0_sb = consts.tile([128, 1], F32)
        nc.sync.dma_start(out=b0_sb, in_=b0_d)
        eps_sb = consts.tile([128, 1], F32)
        nc.vector.memset(eps_sb, EPS)

        xt_sb = xtp.tile([128, NDK, S], BF)
        for kt in range(NDK):
            nc.sync.dma_start(
                out=xt_sb[:, kt, :], in_=xt_d[128 * kt : 128 * (kt + 1), :]
            )

        # ---- persistent activations ----
        qT = persist.tile([128, 2, S], BF)      # (2 heads)*64 rows per m-tile
        kT = persist.tile([128, 2, S], BF)
        vv = persist.tile([128, NKT, HLOC, HD + 1], BF)   # [V | ones]
        at = persist.tile([128, 2, S], BF)      # normalized attn^T
        # pair p's two rows live at partition 32*p (engines need 32-aligned
        # start partitions)
        ln8 = persist.tile([98, NST, 512], F32)
        rr8 = persist.tile([98, NST, 512], BF)
        rr_dram = dscratch.tile([8, NST, 512], BF)
        den_dram = dscratch.tile([4, NST, 512], F32)

        nc.vector.memset(vv[:, :, :, HD : HD + 1], 1.0)

        # ---- phase 1: V projection (s, d) ----
        for st in range(NKT):  # 128-row seq tiles
            ps = vp.tile([128, 256], F32, tag="vp")
            for kt in range(NDK):
                nc.tensor.matmul(
                    ps,
                    lhsT=xt_sb[:, kt, 128 * st : 128 * (st + 1)],
                    rhs=wv_sb[:, kt, :],
                    start=(kt == 0),
                    stop=(kt == NDK - 1),
                )
            nc.vector.tensor_copy(
                vv[:, st, :, 0:HD], ps.rearrange("p (h d) -> p h d", h=HLOC)
            )

        # ---- phase 2: Q/K projection + RMS norm + RoPE, (d, s) layout ----
        for qk_i, (wsb, nsb, dest) in enumerate(
            [(wq_sb, qn_sb, qT), (wk_sb, kn_sb, kT)]
        ):
            for mt in range(2):
                pair = qk_i * 2 + mt
                qraw = work2.tile([128, S], BF, tag="qraw")
                ms = msp.tile([2, NST, 512], F32, tag="msp")
                for st in range(NST):
                    ps = pp.tile([128, 512], F32, tag="pp")
                    for kt in range(NDK):
                        nc.tensor.matmul(
                            ps,
                            lhsT=wsb[:, kt, 128 * mt : 128 * (mt + 1)],
                            rhs=xt_sb[:, kt, 512 * st : 512 * (st + 1)],
                            start=(kt == 0),
                            stop=(kt == NDK - 1),
                        )
                    sl = slice(512 * st, 512 * (st + 1))
                    nc.vector.tensor_copy(qraw[:, sl], ps)
                    sq = sqp.tile([128, 512], BF, tag="sq")
                    nc.vector.tensor_mul(sq, qraw[:, sl], qraw[:, sl])
                    nc.tensor.matmul(
                        ms[:, st, :], lhsT=ones2_sb, rhs=sq, start=True, stop=True
                    )
                # rrms = exp(-0.5 * ln(ms/HD + eps)) for the 2 heads of this pair
                pb = 32 * pair
                nc.scalar.activation(
                    ln8[pb : pb + 2],
                    ms,
                    A.Ln,
                    bias=eps_sb[0:2],
                    scale=1.0 / HD,
                )
                nc.scalar.activation(
                    rr8[pb : pb + 2],
                    ln8[pb : pb + 2],
                    A.Exp,
                    scale=-0.5,
                )
                # broadcast rrms rows across the 64 partitions of each head
                rrb = work2.tile([128, NST, 512], BF, tag="rrb")
                nc.sync.dma_start(
                    out=rr_dram[2 * pair : 2 * pair + 2],
                    in_=rr8[pb : pb + 2],
                )
                nc.gpsimd.dma_start(
                    out=rrb[0:64],
                    in_=rr_dram[2 * pair : 2 * pair + 1].partition_broadcast(64),
                )
                nc.gpsimd.dma_start(
                    out=rrb[64:128],
                    in_=rr_dram[2 * pair + 1 : 2 * pair + 2].partition_broadcast(64),
                )
                # qhat = (qraw * qn_w) * rrms   (normalize + weight, bf16)
                qhat = work2.tile([128, S], BF, tag="qhat")
                for st in range(NST):
                    sl = slice(512 * st, 512 * (st + 1))
                    nc.vector.scalar_tensor_tensor(
                        qhat[:, sl],
                        qraw[:, sl],
                        nsb,
                        rrb[:, st, :],
                        op0=OP.mult,
                        op1=OP.mult,
                    )
                # RoPE: dest = qhat*cos + rot(qhat)*sn  (sn sign-folded on host)
                rot = work2.tile([128, S], BF, tag="rot")
                for lo, hi in ((0, 32), (32, 64), (64, 96), (96, 128)):
                    src_lo = lo + 32 if (lo // 32) % 2 == 0 else lo - 32
                    nc.sync.dma_start(
                        out=rot[lo:hi], in_=qhat[src_lo : src_lo + 32]
                    )
                t1 = work2.tile([128, S], BF, tag="t1")
                t2 = work2.tile([128, S], BF, tag="t2")
                nc.vector.tensor_mul(t1, qhat, cs_sb)
                nc.vector.tensor_mul(t2, rot, sn_sb)
                nc.vector.tensor_add(dest[:, mt, :], t1, t2)

        # proj scratch (incl. x^T) is dead now; free SBUF/PSUM for attention
        proj_ctx.close()
        attn_ctx = ExitStack()
        attnw = ctx.enter_context(tc.tile_pool(name="attnw", bufs=2))
        ptp = ctx.enter_context(tc.tile_pool(name="ptp", bufs=3))
        ystp = ctx.enter_context(tc.tile_pool(name="ystp", bufs=3))
        spp = attn_ctx.enter_context(tc.tile_pool(name="spp", bufs=2, space="PSUM"))
        avp = attn_ctx.enter_context(tc.tile_pool(name="avp", bufs=1, space="PSUM"))

        # ---- phase 3: attention, one head at a time ----
        for h in range(HLOC):
            mt, half = divmod(h, 2)
            po = 64 * half
            av = avp.tile([65, NST, 512], F32, tag="avp")
            for i in range(NKT):
                q0 = 128 * i  # first unmasked query column for this k-tile
                pt = ptp.tile([128, S], BF, tag="pt")
                for g in range(i // 8, 2):
                    sp = spp.tile([128, 1024], F32, tag="spp")
                    glo = 1024 * g
                    has_diag = glo <= q0 < glo + 1024  # only for g == i//8
                    for jj in range(2):
                        j = 2 * g + jj
                        lo = max(512 * j, q0)
                        hi = 512 * (j + 1)
                        if lo >= hi:
                            continue
                        diag_bank = has_diag and (q0 - glo) // 512 == jj
                        nc.tensor.matmul(
                            sp[:, lo - glo : hi - glo],
                            lhsT=kT[po : po + 64, mt, 128 * i : 128 * (i + 1)],
                            rhs=qT[po : po + 64, mt, lo:hi],
                            start=True,
                            stop=not diag_bank,
                        )
                        if diag_bank:
                            # block-causal mask: scores -= 720*disallowed
                            nc.tensor.matmul(
                                sp[:, q0 - glo : q0 - glo + 128],
                                lhsT=mu_sb,
                                rhs=mv_sb,
                                start=False,
                                stop=True,
                            )
                    lo = max(glo, q0)
                    nc.scalar.activation(
                        pt[:, lo : glo + 1024],
                        sp[:, lo - glo : 1024],
                        A.Exp,
                        bias=b0_sb,
                        scale=SCALE,
                    )
                # attn^T accumulation over k-tiles (+ denominator row 64)
                for j in range(i // 4, NST):
                    lo = max(512 * j, q0)
                    hi = 512 * (j + 1)
                    nc.tensor.matmul(
                        av[:, j, lo - 512 * j : 512],
                        lhsT=vv[:, i, h, :],
                        rhs=pt[:, lo:hi],
                        start=(i == 0),
                        stop=(i == 4 * j + 3),
                    )
            # normalize: at[head rows] = av[0:64] * (1 / av[64])
            rden = attnw.tile([1, NST, 512], F32, tag="rden")
            nc.vector.reciprocal(rden, av[64:65])
            nc.sync.dma_start(out=den_dram[h : h + 1], in_=rden)
            rdb = attnw.tile([64, NST, 512], F32, tag="rdb")
            nc.gpsimd.dma_start(
                out=rdb, in_=den_dram[h : h + 1].partition_broadcast(64)
            )
            for j in range(NST):
                nc.vector.tensor_mul(
                    at[po : po + 64, mt, 512 * j : 512 * (j + 1)],
                    av[0:64, j, :],
                    rdb[:, j, :],
                )

        # ---- phase 4: partial out-projection y^T = wo^T @ at ----
        attn_ctx.close()
        pp = ctx.enter_context(tc.tile_pool(name="pp2", bufs=3, space="PSUM"))
        for m in range(8):
            for j in range(NST):
                ps = pp.tile([128, 512], F32, tag="pp")
                for kt in range(2):
                    nc.tensor.matmul(
                        ps,
                        lhsT=wo_sb[:, kt, 128 * m : 128 * (m + 1)],
                        rhs=at[:, kt, 512 * j : 512 * (j + 1)],
                        start=(kt == 0),
                        stop=(kt == 1),
                    )
                yst = ystp.tile([128, 512], F32, tag="yst")
                if (m + j) % 2 == 0:
                    nc.vector.tensor_copy(yst, ps)
                else:
                    nc.scalar.copy(yst, ps)
                nc.sync.dma_start(out=yt_d[4 * m + j], in_=yst)


def _pin_act_table(arch):
    # Force every activation we use (Exp, Ln, Copy) onto the one table set
    # that contains them all, so the program does a single ACT_TABLE_LOAD
    # instead of thrashing natural_log <-> exp_and_others per RMS-norm pair.
    from concourse.hw_specs import get_activation_tables

    tabs = get_activation_tables(arch)
    for nm, s in tabs.items():
        if nm != "natural_log_exp_and_others":
            s.clear()


def build_program():
    nc = bacc.Bacc(
        "TRN2",
        target_bir_lowering=False,
        debug=False,
        enable_asserts=False,
        num_devices=NCORES,
    )
    with tile.TileContext(nc) as tc:
        _emit(tc)
    _pin_act_table(nc.m.arch)
    nc.compile()
    return nc


def make_core_inputs(x, qkv_w, out_w, qn_w, kn_w, rope_cos, rope_sin, attention_mask):
    """Host-side shard/layout prep. Returns list of 8 per-core input dicts."""
    x = np.asarray(x, np.float32)
    qkv_w = np.asarray(qkv_w, np.float32)
    out_w = np.asarray(out_w, np.float32)
    qn_w = np.asarray(qn_w, np.float32)
    kn_w = np.asarray(kn_w, np.float32)
    rope_cos = np.asarray(rope_cos, np.float32)
    rope_sin = np.asarray(rope_sin, np.float32)
    am = np.asarray(attention_mask)

    r = qkv_w.reshape(3, H, HD, D)
    csT = rope_cos.T.astype(BF16)                      # (64, S)
    snT = rope_sin.T.astype(np.float32)
    s2 = np.concatenate([-snT[0:32], snT[32:64]], axis=0).astype(BF16)
    cs2 = np.concatenate([csT, csT], axis=0)           # (128, S)
    sn2 = np.concatenate([s2, s2], axis=0)
    qn2 = np.tile(qn_w, 2)[:, None].astype(np.float32)
    kn2 = np.tile(kn_w, 2)[:, None].astype(np.float32)

    # rank-8 factorization of the (128,128) diagonal-block mask
    dis = ~(am[0:128, 0:128].T)                        # dis[k', q'] disallowed
    mu = np.zeros((8, 128), np.float32)
    mv = np.zeros((8, 128), np.float32)
    for t in range(8):
        mu[t] = np.arange(128) // 16 == t
        mv[t] = -MASK_C * dis[16 * t, :]
    ones2 = np.zeros((128, 2), np.float32)
    ones2[0:64, 0] = 1.0
    ones2[64:128, 1] = 1.0
    b0 = float(HD * SCALE * max(1e-30, np.abs(qn_w).max() * np.abs(kn_w).max()))
    b0_t = np.full((128, 1), -b0, np.float32)

    shared = dict(
        cs=cs2.astype(BF16),
        sn=sn2.astype(BF16),
        qn2=qn2,
        kn2=kn2,
        mu=mu.astype(BF16),
        mv=mv.astype(BF16),
        ones2=ones2.astype(BF16),
        b0=b0_t,
    )
    in_maps = []
    for c in range(NCORES):
        b, g = divmod(c, 4)
        hs = slice(HLOC * g, HLOC * (g + 1))
        m = dict(shared)
        m["xt"] = np.ascontiguousarray(x[b].T).astype(BF16)
        def _wlayout(w):
            # (D, M) -> (128, NDK*M): partition p holds [t, m] = w[t*128+p, m]
            mm = w.shape[1]
            return np.ascontiguousarray(
                w.reshape(-1, 128, mm).transpose(1, 0, 2).reshape(128, -1)
            ).astype(BF16)

        m["wq"] = _wlayout(r[0, hs].transpose(2, 0, 1).reshape(D, 256))
        m["wk"] = _wlayout(r[1, hs].transpose(2, 0, 1).reshape(D, 256))
        m["wv"] = _wlayout(r[2, hs].transpose(2, 0, 1).reshape(D, 256))
        m["wo"] = _wlayout(
            np.ascontiguousarray(out_w[:, 256 * g : 256 * (g + 1)].T)
        )
        in_maps.append(m)
    return in_maps


_PROGRAM = []


def get_program():
    if not _PROGRAM:
        _PROGRAM.append(build_program())
    return _PROGRAM[0]


def unshard(results):
    """results: list of 8 dicts with 'yt' (1024, 2048) fp32 partials."""
    ys = []
    for b in range(B):
        acc = np.zeros((32, 128, 512), np.float64)
        for g in range(4):
            acc += np.asarray(results[4 * b + g]["yt"], np.float32)
        yt = acc.reshape(8, 4, 128, 512).transpose(0, 2, 1, 3).reshape(D, S)
        ys.append(yt.T.astype(np.float32))
    return np.stack(ys)


def kernel(**inputs):
    in_maps = make_core_inputs(**inputs)
    nc = get_program()
    res = run_bass_kernel_spmd(nc, in_maps, core_ids=list(range(NCORES)))
    return unshard(res.results)
